# revision 4
# baseline (speedup 1.0000x reference)
"""Trainium2 Bass kernel for nn_CatEncoderCross.

Computes out[b,i,j,:] = input1[b,i,:] @ W[:768] + input2[b,j,:] @ W[768:] + bias
for shapes input1/input2 [4,128,768], W [1536,768], b [768],
output [4,128,128,768] (f32, ~192 MB).

Sharding: (batch, dout-half): core c handles batch c//2 and output columns
[384*(c%2), 384*(c%2)+384).

All operands stream in bf16 (single plane; tolerance is 2e-2, bf16-only
lands ~1e-3).  Per core:
  p1 = x1 @ W1[:, half]          [128, 384]  (PE, chunk-chased behind DMA)
  p2 = x2 @ W2[:, half] + bias   [128, 384]  (PE, chunk-chased)
  rows 0-31 of p1 broadcast via one-hot selector matmuls (K=32,
  tile_position (0,0)) straight out of p1's natural layout; rows 32-127
  go through the partition-flatten DMA (off critical path) + K=1 ones
  matmuls at tile_position (32b, 0).
  out groups: psum = bcast(p1[i]); out_tile = psum + p2 (DVE); DMA to HBM.
"""

import os
import numpy as np

P = 128
DO = 384  # output columns per core (dout/2)
KO = 6  # K chunks of 128 in d1 (=d2)
NI = 128  # n1 rows per core (full batch)
NJ = 128  # n2
NCORES = 8
FUSE = 3  # output tiles per psum group / DVE op / DMA (steady state)
NSINGLE = 3  # leading single-row groups to start the write stream early

OUT_BUFS = int(os.environ.get("KERNEL_OUT_BUFS", "4"))
PSUM_BUFS = int(os.environ.get("KERNEL_PSUM_BUFS", "2"))
WARM_MMS = int(os.environ.get("KERNEL_WARM_MMS", "26"))

_cache = {}


def _group_rows():
    """Row-group sizes: NSINGLE singles then FUSE-wide, covering NI rows."""
    sizes = [1] * NSINGLE
    left = NI - NSINGLE
    while left > 0:
        take = min(FUSE, left)
        sizes.append(take)
        left -= take
    return sizes


def _build_module():
    import concourse.bacc as bacc
    import concourse.mybir as mybir
    import concourse.tile as tile

    F32 = mybir.dt.float32
    BF16 = mybir.dt.bfloat16

    nc = bacc.Bacc("TRN2", target_bir_lowering=False, debug=False)

    # --- DRAM I/O ---
    x1T_d = nc.dram_tensor("x1T", [P, KO, NI], BF16, kind="ExternalInput")
    x2T_d = nc.dram_tensor("x2T", [P, KO, NJ], BF16, kind="ExternalInput")
    w_d = nc.dram_tensor("Wr", [P, 2 * KO, DO], BF16, kind="ExternalInput")
    bias_d = nc.dram_tensor("biasr", [1, DO], BF16, kind="ExternalInput")
    sel_d = nc.dram_tensor("sel0", [32, 32, P], BF16, kind="ExternalInput")
    out_d = nc.dram_tensor("out", [NI, NJ, DO], F32, kind="ExternalOutput")
    out_ap = out_d.ap()

    with tile.TileContext(nc) as tc:
        with (
            tc.tile_pool(name="const", bufs=1) as cpool,
            tc.tile_pool(name="psum", bufs=PSUM_BUFS, space="PSUM") as pspool,
            tc.tile_pool(name="outp", bufs=OUT_BUFS) as opool,
        ):
            w_sb = [
                cpool.tile([P, DO], BF16, tag=f"w{o}", name=f"w{o}")
                for o in range(2 * KO)
            ]
            x1T_sb = cpool.tile([P, KO, NI], BF16, tag="x1T")
            x2T_sb = cpool.tile([P, KO, NJ], BF16, tag="x2T")
            bias_sb = cpool.tile([1, DO], BF16, tag="bias")
            sel_sb = cpool.tile([32, 32, P], BF16, tag="sel0")
            ones_bf = cpool.tile([97, P], BF16, tag="ones_bf")
            p1bf = cpool.tile([NI, DO], BF16, tag="p1bf")
            p2_sb = cpool.tile([P, DO], F32, tag="p2")
            # p1 rows 32-127 flattened: partition 32b holds rows
            # [32b, 32b+32) of p1 (b = 1..3)
            p1f = cpool.tile([97, 32, DO], BF16, tag="p1f")

            nc.vector.memset(ones_bf[:], 1.0)

            if WARM_MMS:
                warm_ps = pspool.tile([P, 512 * FUSE], F32, tag="ps", name="warm_ps")
                for _ in range(WARM_MMS):
                    nc.tensor.matmul(
                        warm_ps[:, 0:P],
                        ones_bf[0:2, :],
                        ones_bf[0:2, :],
                        start=True,
                        stop=True,
                    )

            # --- input DMAs: bias, x1T, W1 chunks, x2T, W2 chunks, sel ---
            wr = w_d.ap()
            nc.sync.dma_start(out=bias_sb[:], in_=bias_d.ap())
            nc.sync.dma_start(out=x1T_sb[:], in_=x1T_d.ap())
            for k in range(KO):
                nc.sync.dma_start(out=w_sb[k][:], in_=wr[:, k])
            nc.sync.dma_start(out=x2T_sb[:], in_=x2T_d.ap())
            for k in range(KO):
                nc.sync.dma_start(out=w_sb[KO + k][:], in_=wr[:, KO + k])
            nc.sync.dma_start(out=sel_sb[:], in_=sel_d.ap())

            # --- p1 = x1 @ W1 (chunk-chased) ---
            p1_ps = pspool.tile([P, 512 * FUSE], F32, tag="ps", name="p1_ps")
            for k in range(KO):
                nc.tensor.matmul(
                    p1_ps[:, 0:DO],
                    x1T_sb[:, k, :],
                    w_sb[k][:],
                    start=(k == 0),
                    stop=(k == KO - 1),
                )
            nc.vector.tensor_copy(out=p1bf[:], in_=p1_ps[:, 0:DO])

            # --- flatten p1 rows 32-127 for the K=1 broadcast matmuls ---
            for b in range(1, 4):
                nc.sync.dma_start(
                    out=p1f[32 * b : 32 * b + 1],
                    in_=p1bf[32 * b : 32 * b + 32, :],
                )

            # --- p2 = x2 @ W2 + bias (chunk-chased; bias lands first) ---
            p2_ps = pspool.tile([P, 512 * FUSE], F32, tag="ps", name="p2_ps")
            nc.tensor.matmul(
                p2_ps[:, 0:DO],
                ones_bf[0:1, :],
                bias_sb[:],
                start=True,
                stop=False,
                tile_position=(0, 0),
            )
            for k in range(KO):
                nc.tensor.matmul(
                    p2_ps[:, 0:DO],
                    x2T_sb[:, k, :],
                    w_sb[KO + k][:],
                    start=False,
                    stop=(k == KO - 1),
                )
            nc.vector.tensor_copy(out=p2_sb[:], in_=p2_ps[:, 0:DO])

            def bcast_mm(ps, m, i):
                """psum[:, 512m:512m+DO] = broadcast of p1 row i."""
                b = i // 32
                if b == 0:
                    nc.tensor.matmul(
                        ps[:, 512 * m : 512 * m + DO],
                        sel_sb[:, i, :],
                        p1bf[0:32, :],
                        start=True,
                        stop=True,
                        tile_position=(0, 0),
                    )
                else:
                    nc.tensor.matmul(
                        ps[:, 512 * m : 512 * m + DO],
                        ones_bf[32 * b : 32 * b + 1, :],
                        p1f[32 * b : 32 * b + 1, i % 32, :],
                        start=True,
                        stop=True,
                        tile_position=(32 * b, 0),
                    )

            # --- main loop over row groups ---
            i0 = 0
            for gi, rows in enumerate(_group_rows()):
                ob = opool.tile([P, rows, DO], F32, tag="ob", name=f"ob{gi}")
                ps = pspool.tile([P, 512 * FUSE], F32, tag="ps", name=f"ps{gi}")
                for m in range(rows):
                    bcast_mm(ps, m, i0 + m)
                ps_v = ps.rearrange("p (i x) -> p i x", i=FUSE)[:, 0:rows, 0:DO]
                if rows == 1:
                    nc.vector.tensor_add(
                        out=ob[:, 0, :], in0=ps_v[:, 0, :], in1=p2_sb[:]
                    )
                else:
                    p2_b = p2_sb[:, None, :].to_broadcast((P, rows, DO))
                    nc.vector.tensor_add(out=ob[:], in0=ps_v, in1=p2_b)
                dst = out_ap[i0 : i0 + rows]  # [rows, NJ, DO]
                nc.sync.dma_start(out=dst.rearrange("i j d -> j i d"), in_=ob[:])
                i0 += rows

    nc.compile()
    return nc


def _get_module():
    key = (OUT_BUFS, PSUM_BUFS, WARM_MMS)
    if key not in _cache:
        _cache[key] = _build_module()
    return _cache[key]


def _to_bf16(x):
    import ml_dtypes

    return x.astype(ml_dtypes.bfloat16)


def _prep_xT(x):
    """[128, 768] f32 -> [128, KO, 128] bf16 transposed chunk layout."""
    return np.ascontiguousarray(
        _to_bf16(x).T.reshape(KO, P, P).transpose(1, 0, 2)
    )


def _make_in_maps(input1, input2, W, b):
    import ml_dtypes

    input1 = np.asarray(input1, dtype=np.float32)
    input2 = np.asarray(input2, dtype=np.float32)
    W = np.asarray(W, dtype=np.float32)
    b = np.asarray(b, dtype=np.float32)

    sel0 = np.ascontiguousarray(
        np.broadcast_to(np.eye(32, dtype=np.float32)[:, :, None], (32, 32, P))
    ).astype(ml_dtypes.bfloat16)

    in_maps = []
    for c in range(NCORES):
        bb, h = divmod(c, 2)
        Whalf = _to_bf16(W[:, h * DO : (h + 1) * DO])
        Wr = np.ascontiguousarray(
            Whalf.reshape(2 * KO, P, DO).transpose(1, 0, 2)
        )
        biasr = np.ascontiguousarray(_to_bf16(b[h * DO : (h + 1) * DO])[None])
        in_maps.append(
            {
                "x1T": _prep_xT(input1[bb]),
                "x2T": _prep_xT(input2[bb]),
                "Wr": Wr,
                "biasr": biasr,
                "sel0": sel0,
            }
        )
    return in_maps


def kernel(input1, input2, W, b):
    from concourse import bass_utils

    suppress_trace = False
    if os.environ.get("BASS_TRACE"):
        try:
            from antenv.axon_hooks import get_axon_ntff_profile_hook  # noqa: F401
        except Exception:
            suppress_trace = True
    prev = os.environ.get("BASS_NEVER_TRACE")
    if suppress_trace:
        os.environ["BASS_NEVER_TRACE"] = "1"
    try:
        nc = _get_module()
        in_maps = _make_in_maps(input1, input2, W, b)
        res = bass_utils.run_bass_kernel_spmd(
            nc, in_maps, core_ids=list(range(NCORES))
        )
    finally:
        if suppress_trace:
            if prev is None:
                os.environ.pop("BASS_NEVER_TRACE", None)
            else:
                os.environ["BASS_NEVER_TRACE"] = prev
    out = np.empty((4, NJ, NJ, 2 * DO), dtype=np.float32)
    for c in range(NCORES):
        bb, h = divmod(c, 2)
        out[bb, :, :, h * DO : (h + 1) * DO] = res.results[c]["out"]
    return out


# revision 8
# speedup vs baseline: 1.0267x; 1.0267x over previous
"""Trainium2 Bass kernel for nn_CatEncoderCross.

Computes out[b,i,j,:] = input1[b,i,:] @ W[:768] + input2[b,j,:] @ W[768:] + bias
for shapes input1/input2 [4,128,768], W [1536,768], b [768],
output [4,128,128,768] (f32, ~192 MB).

Sharding: (batch, dout-half): core c handles batch c//2 and output columns
[384*(c%2), 384*(c%2)+384).

All operands stream in bf16 (single plane; tolerance is 2e-2, bf16-only
lands ~3e-3).  DMA issuance costs ~650ns SEQ+HWDGE per instruction, so
inputs ride in two big merged DMAs (x1T|W1, x2T|W2).  Per core:
  p1 = x1 @ W1[:, half]          [128, 384]  (PE)
  p2 = x2 @ W2[:, half] + bias   [128, 384]  (PE, stays in PSUM; the DVE
       adds read it straight from the PSUM bank)
  rows 0-11 of p1 broadcast via one-hot selector matmuls (K=32,
  tile_position (0,0)) straight out of p1's natural layout — no flatten
  DMA on the critical path; rows 12-127 use the partition-flatten DMAs
  (Activation-engine queue, off critical path) + K=1 ones matmuls.
  out groups: psum = bcast(p1[i]); out_tile = psum + p2 (DVE); DMA to HBM.
"""

import os
import numpy as np

P = 128
DO = 384  # output columns per core (dout/2)
KO = 6  # K chunks of 128 in d1 (=d2)
NI = 128  # n1 rows per core (full batch)
NJ = 128  # n2
NCORES = 8
FUSE = 3  # output tiles per psum group / DVE op / DMA (steady state)
NSINGLE = 3  # leading single-row groups to start the write stream early
SELR = 12  # p1 rows broadcast via the selector (rest via flatten)

OUT_BUFS = int(os.environ.get("KERNEL_OUT_BUFS", "4"))
PSUM_BUFS = int(os.environ.get("KERNEL_PSUM_BUFS", "2"))
WARM_MMS = int(os.environ.get("KERNEL_WARM_MMS", "30"))

_cache = {}


def _group_rows():
    """Row-group sizes: NSINGLE singles then FUSE-wide, covering NI rows."""
    sizes = [1] * NSINGLE
    left = NI - NSINGLE
    if left % FUSE:
        sizes.append(left % FUSE)
        left -= left % FUSE
    sizes += [FUSE] * (left // FUSE)
    return sizes


def _build_module():
    import concourse.bacc as bacc
    import concourse.mybir as mybir
    import concourse.tile as tile

    F32 = mybir.dt.float32
    BF16 = mybir.dt.bfloat16

    nc = bacc.Bacc("TRN2", target_bir_lowering=False, debug=False)

    # --- DRAM I/O (x and W merged per stream: [P, 6*128 + 6*384]) ---
    SW = KO * P + KO * DO
    s1_d = nc.dram_tensor("s1", [P, SW], BF16, kind="ExternalInput")
    s2_d = nc.dram_tensor("s2", [P, SW], BF16, kind="ExternalInput")
    aug_d = nc.dram_tensor("aug", [32, P + DO], BF16, kind="ExternalInput")
    sel_d = nc.dram_tensor("sel0", [32, SELR, P], BF16, kind="ExternalInput")
    out_d = nc.dram_tensor("out", [NI, NJ, DO], F32, kind="ExternalOutput")
    out_ap = out_d.ap()

    with tile.TileContext(nc) as tc:
        with (
            tc.tile_pool(name="const", bufs=1) as cpool,
            tc.tile_pool(name="psum", bufs=PSUM_BUFS, space="PSUM") as pspool,
            tc.tile_pool(name="psum_p", bufs=1, space="PSUM") as pppool,
            tc.tile_pool(name="outp", bufs=OUT_BUFS) as opool,
        ):
            s1_sb = cpool.tile([P, SW], BF16, tag="s1")
            s2_sb = cpool.tile([P, SW], BF16, tag="s2")
            aug_sb = cpool.tile([32, P + DO], BF16, tag="aug")
            sel_sb = cpool.tile([32, SELR, P], BF16, tag="sel0")
            ones_bf = cpool.tile([97, P], BF16, tag="ones_bf")
            p1bf = cpool.tile([NI, DO], BF16, tag="p1bf")
            # p1 flattened: partition 32b holds rows [32b, 32b+32) (b=0..3)
            p1f = cpool.tile([97, 32, DO], BF16, tag="p1f")

            def xk(s, k):  # x chunk k: [P, 128]
                return s[:, P * k : P * (k + 1)]

            def wk(s, k):  # W chunk k: [P, 384]
                return s[:, KO * P + DO * k : KO * P + DO * (k + 1)]

            nc.vector.memset(ones_bf[:], 1.0)

            p1_ps = pppool.tile([P, 512], F32, tag="p1ps", name="p1_ps")
            p2_ps = pppool.tile([P, 512], F32, tag="p2ps", name="p2_ps")

            if WARM_MMS:
                warm_ps = pspool.tile([P, 512 * FUSE], F32, tag="ps", name="warm_ps")
                for _ in range(WARM_MMS):
                    nc.tensor.matmul(
                        warm_ps[:, 0:P],
                        ones_bf[0:2, :],
                        ones_bf[0:2, :],
                        start=True,
                        stop=True,
                    )

            # --- input DMAs ---
            nc.sync.dma_start(out=s1_sb[:], in_=s1_d.ap())
            nc.sync.dma_start(out=aug_sb[:], in_=aug_d.ap())
            nc.sync.dma_start(out=s2_sb[:], in_=s2_d.ap())
            nc.sync.dma_start(out=sel_sb[:], in_=sel_d.ap())

            # --- p1 = x1 @ W1 ---
            for k in range(KO):
                nc.tensor.matmul(
                    p1_ps[:, 0:DO],
                    xk(s1_sb, k),
                    wk(s1_sb, k),
                    start=(k == 0),
                    stop=(k == KO - 1),
                )
            nc.vector.tensor_copy(out=p1bf[:], in_=p1_ps[:, 0:DO])

            # --- flatten p1 for the K=1 broadcast matmuls (Act queue) ---
            for b in range(4):
                nc.scalar.dma_start(
                    out=p1f[32 * b : 32 * b + 1],
                    in_=p1bf[32 * b : 32 * b + 32, :],
                )

            def bcast_mm(ps, m, i):
                """psum[:, 512m:512m+DO] = broadcast of p1 row i."""
                b = i // 32
                if i < SELR:
                    nc.tensor.matmul(
                        ps[:, 512 * m : 512 * m + DO],
                        sel_sb[:, i, :],
                        p1bf[0:32, :],
                        start=True,
                        stop=True,
                        tile_position=(0, 0),
                    )
                else:
                    nc.tensor.matmul(
                        ps[:, 512 * m : 512 * m + DO],
                        ones_bf[32 * b : 32 * b + 1, :],
                        p1f[32 * b : 32 * b + 1, i % 32, :],
                        start=True,
                        stop=True,
                        tile_position=(32 * b, 0),
                    )

            # g0/g1 broadcast matmuls go before the p2 matmuls on PE so
            # they run in the PE idle window while s2 streams in.
            groups = _group_rows()
            g_ps = []
            for gi in range(2):
                ps = pspool.tile([P, 512 * FUSE], F32, tag="ps", name=f"ps{gi}")
                g_ps.append(ps)
                bcast_mm(ps, 0, gi)  # groups 0/1 are single-row

            # --- p2 = x2 @ W2 + bias (bias rides a host-packed K=32
            # augmented chunk: aug[0,:P]=1, aug[0,P:]=bias, rows 1-31 zero;
            # the whole accumulation group is emitted contiguously) ---
            nc.tensor.matmul(
                p2_ps[:, 0:DO],
                aug_sb[:, 0:P],
                aug_sb[:, P : P + DO],
                start=True,
                stop=False,
                tile_position=(0, 0),
            )
            for k in range(KO):
                nc.tensor.matmul(
                    p2_ps[:, 0:DO],
                    xk(s2_sb, k),
                    wk(s2_sb, k),
                    start=False,
                    stop=(k == KO - 1),
                )
            p2_sb = cpool.tile([P, DO], F32, tag="p2")
            nc.scalar.mul(p2_sb[:], p2_ps[:, 0:DO], 1.0)
            p2v = p2_sb[:]

            # --- main loop over row groups ---
            i0 = 0
            for gi, rows in enumerate(groups):
                ob = opool.tile([P, rows, DO], F32, tag="ob", name=f"ob{gi}")
                if gi < 2:
                    ps = g_ps[gi]
                else:
                    ps = pspool.tile(
                        [P, 512 * FUSE], F32, tag="ps", name=f"ps{gi}"
                    )
                    for m in range(rows):
                        bcast_mm(ps, m, i0 + m)
                ps_v = ps.rearrange("p (i x) -> p i x", i=FUSE)[:, 0:rows, 0:DO]
                if rows == 1:
                    nc.vector.tensor_add(
                        out=ob[:, 0, :], in0=ps_v[:, 0, :], in1=p2v
                    )
                else:
                    p2_b = p2v[:, None, :].to_broadcast((P, rows, DO))
                    nc.vector.tensor_add(out=ob[:], in0=ps_v, in1=p2_b)
                dst = out_ap[i0 : i0 + rows]  # [rows, NJ, DO]
                nc.sync.dma_start(out=dst.rearrange("i j d -> j i d"), in_=ob[:])
                i0 += rows

    nc.compile()
    return nc


def _get_module():
    key = (OUT_BUFS, PSUM_BUFS, WARM_MMS)
    if key not in _cache:
        _cache[key] = _build_module()
    return _cache[key]


def _to_bf16(x):
    import ml_dtypes

    return x.astype(ml_dtypes.bfloat16)


def _prep_stream(x, Whalf):
    """x [128,768] f32, Whalf [768,384] f32 -> [128, 6*128+6*384] bf16."""
    xT = _to_bf16(x).T.reshape(KO, P, P).transpose(1, 0, 2).reshape(P, KO * P)
    Wr = (
        _to_bf16(Whalf)
        .reshape(KO, P, DO)
        .transpose(1, 0, 2)
        .reshape(P, KO * DO)
    )
    return np.ascontiguousarray(np.concatenate([xT, Wr], axis=1))


def _make_in_maps(input1, input2, W, b):
    import ml_dtypes

    input1 = np.asarray(input1, dtype=np.float32)
    input2 = np.asarray(input2, dtype=np.float32)
    W = np.asarray(W, dtype=np.float32)
    b = np.asarray(b, dtype=np.float32)

    sel0 = np.ascontiguousarray(
        np.broadcast_to(
            np.eye(32, SELR, dtype=np.float32)[:, :, None], (32, SELR, P)
        )
    ).astype(ml_dtypes.bfloat16)

    in_maps = []
    for c in range(NCORES):
        bb, h = divmod(c, 2)
        W1 = W[:KO * P, h * DO : (h + 1) * DO]
        W2 = W[KO * P :, h * DO : (h + 1) * DO]
        aug = np.zeros((32, P + DO), dtype=np.float32)
        aug[0, :P] = 1.0
        aug[0, P:] = b[h * DO : (h + 1) * DO]
        in_maps.append(
            {
                "s1": _prep_stream(input1[bb], W1),
                "s2": _prep_stream(input2[bb], W2),
                "aug": np.ascontiguousarray(_to_bf16(aug)),
                "sel0": sel0,
            }
        )
    return in_maps


def kernel(input1, input2, W, b):
    from concourse import bass_utils

    suppress_trace = False
    if os.environ.get("BASS_TRACE"):
        try:
            from antenv.axon_hooks import get_axon_ntff_profile_hook  # noqa: F401
        except Exception:
            suppress_trace = True
    prev = os.environ.get("BASS_NEVER_TRACE")
    if suppress_trace:
        os.environ["BASS_NEVER_TRACE"] = "1"
    try:
        nc = _get_module()
        in_maps = _make_in_maps(input1, input2, W, b)
        res = bass_utils.run_bass_kernel_spmd(
            nc, in_maps, core_ids=list(range(NCORES))
        )
    finally:
        if suppress_trace:
            if prev is None:
                os.environ.pop("BASS_NEVER_TRACE", None)
            else:
                os.environ["BASS_NEVER_TRACE"] = prev
    out = np.empty((4, NJ, NJ, 2 * DO), dtype=np.float32)
    for c in range(NCORES):
        bb, h = divmod(c, 2)
        out[bb, :, :, h * DO : (h + 1) * DO] = res.results[c]["out"]
    return out


# revision 9
# speedup vs baseline: 1.0280x; 1.0012x over previous
"""Trainium2 Bass kernel for nn_CatEncoderCross.

Computes out[b,i,j,:] = input1[b,i,:] @ W[:768] + input2[b,j,:] @ W[768:] + bias
for shapes input1/input2 [4,128,768], W [1536,768], b [768],
output [4,128,128,768] (f32, ~192 MB).

Sharding: (batch, dout-half): core c handles batch c//2 and output columns
[384*(c%2), 384*(c%2)+384).

All operands stream in bf16 (single plane; tolerance is 2e-2, bf16-only
lands ~3e-3).  DMA issuance costs ~650ns SEQ+HWDGE per instruction, so
inputs ride in two big merged DMAs (x1T|W1, x2T|W2).  Per core:
  p1 = x1 @ W1[:, half]          [128, 384]  (PE)
  p2 = x2 @ W2[:, half] + bias   [128, 384]  (PE, stays in PSUM; the DVE
       adds read it straight from the PSUM bank)
  rows 0-11 of p1 broadcast via one-hot selector matmuls (K=32,
  tile_position (0,0)) straight out of p1's natural layout — no flatten
  DMA on the critical path; rows 12-127 use the partition-flatten DMAs
  (Activation-engine queue, off critical path) + K=1 ones matmuls.
  out groups: psum = bcast(p1[i]); out_tile = psum + p2 (DVE); DMA to HBM.
"""

import os
import numpy as np

P = 128
DO = 384  # output columns per core (dout/2)
KO = 6  # K chunks of 128 in d1 (=d2)
NI = 128  # n1 rows per core (full batch)
NJ = 128  # n2
NCORES = 8
FUSE = 3  # output tiles per psum group / DVE op / DMA (steady state)
NSINGLE = 3  # leading single-row groups to start the write stream early
SELR = 12  # p1 rows broadcast via the selector (rest via flatten)

OUT_BUFS = int(os.environ.get("KERNEL_OUT_BUFS", "4"))
PSUM_BUFS = int(os.environ.get("KERNEL_PSUM_BUFS", "2"))
WARM_MMS = int(os.environ.get("KERNEL_WARM_MMS", "12"))

_cache = {}


def _group_rows():
    """Row-group sizes: NSINGLE singles then FUSE-wide, covering NI rows."""
    sizes = [1] * NSINGLE
    left = NI - NSINGLE
    if left % FUSE:
        sizes.append(left % FUSE)
        left -= left % FUSE
    sizes += [FUSE] * (left // FUSE)
    return sizes


def _build_module():
    import concourse.bacc as bacc
    import concourse.mybir as mybir
    import concourse.tile as tile

    F32 = mybir.dt.float32
    BF16 = mybir.dt.bfloat16

    nc = bacc.Bacc("TRN2", target_bir_lowering=False, debug=False)

    # --- DRAM I/O (x and W merged per stream: [P, 6*128 + 6*384]) ---
    SW = KO * P + KO * DO
    s1_d = nc.dram_tensor("s1", [P, SW], BF16, kind="ExternalInput")
    s2_d = nc.dram_tensor("s2", [P, SW], BF16, kind="ExternalInput")
    aug_d = nc.dram_tensor("aug", [32, P + DO], BF16, kind="ExternalInput")
    sel_d = nc.dram_tensor("sel0", [32, SELR, P], BF16, kind="ExternalInput")
    out_d = nc.dram_tensor("out", [NI, NJ, DO], F32, kind="ExternalOutput")
    out_ap = out_d.ap()

    with tile.TileContext(nc) as tc:
        with (
            tc.tile_pool(name="const", bufs=1) as cpool,
            tc.tile_pool(name="psum", bufs=PSUM_BUFS, space="PSUM") as pspool,
            tc.tile_pool(name="psum_p", bufs=1, space="PSUM") as pppool,
            tc.tile_pool(name="outp", bufs=OUT_BUFS) as opool,
        ):
            s1_sb = cpool.tile([P, SW], BF16, tag="s1")
            s2_sb = cpool.tile([P, SW], BF16, tag="s2")
            aug_sb = cpool.tile([32, P + DO], BF16, tag="aug")
            sel_sb = cpool.tile([32, SELR, P], BF16, tag="sel0")
            ones_bf = cpool.tile([97, P], BF16, tag="ones_bf")
            p1bf = cpool.tile([NI, DO], BF16, tag="p1bf")
            # p1 flattened: partition 32b holds rows [32b, 32b+32) (b=0..3)
            p1f = cpool.tile([97, 32, DO], BF16, tag="p1f")

            def xk(s, k):  # x chunk k: [P, 128]
                return s[:, P * k : P * (k + 1)]

            def wk(s, k):  # W chunk k: [P, 384]
                return s[:, KO * P + DO * k : KO * P + DO * (k + 1)]

            nc.vector.memset(ones_bf[:], 1.0)

            p1_ps = pppool.tile([P, 512], F32, tag="p1ps", name="p1_ps")
            p2_ps = pppool.tile([P, 512], F32, tag="p2ps", name="p2_ps")

            if WARM_MMS:
                warm_ps = pspool.tile([P, 512 * FUSE], F32, tag="ps", name="warm_ps")
                for _ in range(WARM_MMS):
                    nc.tensor.matmul(
                        warm_ps[:, 0:512],
                        ones_bf[0:2, :].to_broadcast((2, 512)),
                        ones_bf[0:2, :].to_broadcast((2, 512)),
                        start=True,
                        stop=True,
                    )

            # --- input DMAs ---
            nc.sync.dma_start(out=s1_sb[:], in_=s1_d.ap())
            nc.sync.dma_start(out=aug_sb[:], in_=aug_d.ap())
            nc.sync.dma_start(out=s2_sb[:], in_=s2_d.ap())
            nc.sync.dma_start(out=sel_sb[:], in_=sel_d.ap())

            # --- p1 = x1 @ W1 ---
            for k in range(KO):
                nc.tensor.matmul(
                    p1_ps[:, 0:DO],
                    xk(s1_sb, k),
                    wk(s1_sb, k),
                    start=(k == 0),
                    stop=(k == KO - 1),
                )
            nc.vector.tensor_copy(out=p1bf[:], in_=p1_ps[:, 0:DO])

            # --- flatten p1 for the K=1 broadcast matmuls (Act queue) ---
            for b in range(4):
                nc.scalar.dma_start(
                    out=p1f[32 * b : 32 * b + 1],
                    in_=p1bf[32 * b : 32 * b + 32, :],
                )

            def bcast_mm(ps, m, i):
                """psum[:, 512m:512m+DO] = broadcast of p1 row i."""
                b = i // 32
                if i < SELR:
                    nc.tensor.matmul(
                        ps[:, 512 * m : 512 * m + DO],
                        sel_sb[:, i, :],
                        p1bf[0:32, :],
                        start=True,
                        stop=True,
                        tile_position=(0, 0),
                    )
                else:
                    nc.tensor.matmul(
                        ps[:, 512 * m : 512 * m + DO],
                        ones_bf[32 * b : 32 * b + 1, :],
                        p1f[32 * b : 32 * b + 1, i % 32, :],
                        start=True,
                        stop=True,
                        tile_position=(32 * b, 0),
                    )

            # g0/g1 broadcast matmuls go before the p2 matmuls on PE so
            # they run in the PE idle window while s2 streams in.
            groups = _group_rows()
            g_ps = []
            for gi in range(2):
                ps = pspool.tile([P, 512 * FUSE], F32, tag="ps", name=f"ps{gi}")
                g_ps.append(ps)
                bcast_mm(ps, 0, gi)  # groups 0/1 are single-row

            # --- p2 = x2 @ W2 + bias (bias rides a host-packed K=32
            # augmented chunk: aug[0,:P]=1, aug[0,P:]=bias, rows 1-31 zero;
            # the whole accumulation group is emitted contiguously) ---
            nc.tensor.matmul(
                p2_ps[:, 0:DO],
                aug_sb[:, 0:P],
                aug_sb[:, P : P + DO],
                start=True,
                stop=False,
                tile_position=(0, 0),
            )
            for k in range(KO):
                nc.tensor.matmul(
                    p2_ps[:, 0:DO],
                    xk(s2_sb, k),
                    wk(s2_sb, k),
                    start=False,
                    stop=(k == KO - 1),
                )
            p2_sb = cpool.tile([P, DO], F32, tag="p2")
            nc.scalar.mul(p2_sb[:], p2_ps[:, 0:DO], 1.0)
            p2v = p2_sb[:]

            # --- main loop over row groups ---
            i0 = 0
            for gi, rows in enumerate(groups):
                ob = opool.tile([P, rows, DO], F32, tag="ob", name=f"ob{gi}")
                if gi < 2:
                    ps = g_ps[gi]
                else:
                    ps = pspool.tile(
                        [P, 512 * FUSE], F32, tag="ps", name=f"ps{gi}"
                    )
                    for m in range(rows):
                        bcast_mm(ps, m, i0 + m)
                ps_v = ps.rearrange("p (i x) -> p i x", i=FUSE)[:, 0:rows, 0:DO]
                if rows == 1:
                    nc.vector.tensor_add(
                        out=ob[:, 0, :], in0=ps_v[:, 0, :], in1=p2v
                    )
                else:
                    p2_b = p2v[:, None, :].to_broadcast((P, rows, DO))
                    nc.vector.tensor_add(out=ob[:], in0=ps_v, in1=p2_b)
                dst = out_ap[i0 : i0 + rows]  # [rows, NJ, DO]
                nc.sync.dma_start(out=dst.rearrange("i j d -> j i d"), in_=ob[:])
                i0 += rows

    nc.compile()
    return nc


def _get_module():
    key = (OUT_BUFS, PSUM_BUFS, WARM_MMS)
    if key not in _cache:
        _cache[key] = _build_module()
    return _cache[key]


def _to_bf16(x):
    import ml_dtypes

    return x.astype(ml_dtypes.bfloat16)


def _prep_stream(x, Whalf):
    """x [128,768] f32, Whalf [768,384] f32 -> [128, 6*128+6*384] bf16."""
    xT = _to_bf16(x).T.reshape(KO, P, P).transpose(1, 0, 2).reshape(P, KO * P)
    Wr = (
        _to_bf16(Whalf)
        .reshape(KO, P, DO)
        .transpose(1, 0, 2)
        .reshape(P, KO * DO)
    )
    return np.ascontiguousarray(np.concatenate([xT, Wr], axis=1))


def _make_in_maps(input1, input2, W, b):
    import ml_dtypes

    input1 = np.asarray(input1, dtype=np.float32)
    input2 = np.asarray(input2, dtype=np.float32)
    W = np.asarray(W, dtype=np.float32)
    b = np.asarray(b, dtype=np.float32)

    sel0 = np.ascontiguousarray(
        np.broadcast_to(
            np.eye(32, SELR, dtype=np.float32)[:, :, None], (32, SELR, P)
        )
    ).astype(ml_dtypes.bfloat16)

    in_maps = []
    for c in range(NCORES):
        bb, h = divmod(c, 2)
        W1 = W[:KO * P, h * DO : (h + 1) * DO]
        W2 = W[KO * P :, h * DO : (h + 1) * DO]
        aug = np.zeros((32, P + DO), dtype=np.float32)
        aug[0, :P] = 1.0
        aug[0, P:] = b[h * DO : (h + 1) * DO]
        in_maps.append(
            {
                "s1": _prep_stream(input1[bb], W1),
                "s2": _prep_stream(input2[bb], W2),
                "aug": np.ascontiguousarray(_to_bf16(aug)),
                "sel0": sel0,
            }
        )
    return in_maps


def kernel(input1, input2, W, b):
    from concourse import bass_utils

    suppress_trace = False
    if os.environ.get("BASS_TRACE"):
        try:
            from antenv.axon_hooks import get_axon_ntff_profile_hook  # noqa: F401
        except Exception:
            suppress_trace = True
    prev = os.environ.get("BASS_NEVER_TRACE")
    if suppress_trace:
        os.environ["BASS_NEVER_TRACE"] = "1"
    try:
        nc = _get_module()
        in_maps = _make_in_maps(input1, input2, W, b)
        res = bass_utils.run_bass_kernel_spmd(
            nc, in_maps, core_ids=list(range(NCORES))
        )
    finally:
        if suppress_trace:
            if prev is None:
                os.environ.pop("BASS_NEVER_TRACE", None)
            else:
                os.environ["BASS_NEVER_TRACE"] = prev
    out = np.empty((4, NJ, NJ, 2 * DO), dtype=np.float32)
    for c in range(NCORES):
        bb, h = divmod(c, 2)
        out[bb, :, :, h * DO : (h + 1) * DO] = res.results[c]["out"]
    return out


# revision 12
# speedup vs baseline: 1.0433x; 1.0149x over previous
"""Trainium2 Bass kernel for nn_CatEncoderCross.

Computes out[b,i,j,:] = input1[b,i,:] @ W[:768] + input2[b,j,:] @ W[768:] + bias
for shapes input1/input2 [4,128,768], W [1536,768], b [768],
output [4,128,128,768] (f32, ~192 MB).

Sharding: (batch, dout-half): core c handles batch c//2 and output columns
[384*(c%2), 384*(c%2)+384).

All operands stream in bf16 (single plane; tolerance is 2e-2, bf16-only
lands ~3e-3).  DMA issuance costs ~650ns SEQ+HWDGE per instruction, so
inputs ride in two big merged DMAs (x1T|W1, x2T|W2).  Per core:
  p1 = x1 @ W1[:, half]          [128, 384]  (PE)
  p2 = x2 @ W2[:, half] + bias   [128, 384]  (PE, stays in PSUM; the DVE
       adds read it straight from the PSUM bank)
  rows 0-11 of p1 broadcast via one-hot selector matmuls (K=32,
  tile_position (0,0)) straight out of p1's natural layout — no flatten
  DMA on the critical path; rows 12-127 use the partition-flatten DMAs
  (Activation-engine queue, off critical path) + K=1 ones matmuls.
  out groups: psum = bcast(p1[i]); out_tile = psum + p2 (DVE); DMA to HBM.
"""

import os
import numpy as np

P = 128
DO = 384  # output columns per core (dout/2)
KO = 6  # K chunks of 128 in d1 (=d2)
NI = 128  # n1 rows per core (full batch)
NJ = 128  # n2
NCORES = 8
FUSE = 3  # output tiles per psum group / DVE op / DMA (steady state)
NSINGLE = 3  # leading single-row groups to start the write stream early
SELR = 12  # p1 rows broadcast via the selector (rest via flatten)

OUT_BUFS = int(os.environ.get("KERNEL_OUT_BUFS", "4"))
PSUM_BUFS = int(os.environ.get("KERNEL_PSUM_BUFS", "2"))
WARM_MMS = int(os.environ.get("KERNEL_WARM_MMS", "8"))

_cache = {}


def _group_rows():
    """Row-group sizes: NSINGLE singles then FUSE-wide, covering NI rows."""
    sizes = [1] * NSINGLE
    left = NI - NSINGLE
    if left % FUSE:
        sizes.append(left % FUSE)
        left -= left % FUSE
    sizes += [FUSE] * (left // FUSE)
    return sizes


def _build_module():
    import concourse.bacc as bacc
    import concourse.mybir as mybir
    import concourse.tile as tile

    F32 = mybir.dt.float32
    BF16 = mybir.dt.bfloat16

    nc = bacc.Bacc("TRN2", target_bir_lowering=False, debug=False)

    # --- DRAM I/O (x and W merged per stream: [P, 6*128 + 6*384]) ---
    SW = KO * P + KO * DO
    s1_d = nc.dram_tensor("s1", [P, SW], BF16, kind="ExternalInput")
    s2_d = nc.dram_tensor("s2", [P, SW], BF16, kind="ExternalInput")
    aug_d = nc.dram_tensor("aug", [32, P + DO], BF16, kind="ExternalInput")
    sel_d = nc.dram_tensor("sel0", [32, SELR, P], BF16, kind="ExternalInput")
    out_d = nc.dram_tensor("out", [NI, NJ, DO], F32, kind="ExternalOutput")
    out_ap = out_d.ap()

    with tile.TileContext(nc) as tc:
        with (
            tc.tile_pool(name="const", bufs=1) as cpool,
            tc.tile_pool(name="psum", bufs=PSUM_BUFS, space="PSUM") as pspool,
            tc.tile_pool(name="psum_p", bufs=1, space="PSUM") as pppool,
            tc.tile_pool(name="outp", bufs=OUT_BUFS) as opool,
        ):
            s1_sb = cpool.tile([P, SW], BF16, tag="s1")
            s2_sb = cpool.tile([P, SW], BF16, tag="s2")
            aug_sb = cpool.tile([32, P + DO], BF16, tag="aug")
            sel_sb = cpool.tile([32, SELR, P], BF16, tag="sel0")
            ones_bf = cpool.tile([97, 512], BF16, tag="ones_bf")
            p1bf = cpool.tile([NI, DO], BF16, tag="p1bf")
            # p1 flattened: partition 32b holds rows [32b, 32b+32) (b=0..3)
            p1f = cpool.tile([97, 32, DO], BF16, tag="p1f")

            def xk(s, k):  # x chunk k: [P, 128]
                return s[:, P * k : P * (k + 1)]

            def wk(s, k):  # W chunk k: [P, 384]
                return s[:, KO * P + DO * k : KO * P + DO * (k + 1)]

            nc.vector.memset(ones_bf[:], 1.0)

            p1_ps = pppool.tile([P, 512], F32, tag="p1ps", name="p1_ps")
            p2_ps = pppool.tile([P, 512], F32, tag="p2ps", name="p2_ps")

            if WARM_MMS:
                warm_ps = pspool.tile([P, 512 * FUSE], F32, tag="ps", name="warm_ps")
                for _ in range(WARM_MMS):
                    nc.tensor.matmul(
                        warm_ps[:, 0:512],
                        ones_bf[0:2, 0:P],
                        ones_bf[0:2, 0:512],
                        start=True,
                        stop=True,
                    )

            # --- input DMAs ---
            nc.sync.dma_start(out=s1_sb[:], in_=s1_d.ap())
            nc.sync.dma_start(out=aug_sb[:], in_=aug_d.ap())
            nc.sync.dma_start(out=s2_sb[:], in_=s2_d.ap())
            nc.sync.dma_start(out=sel_sb[:], in_=sel_d.ap())

            # --- p1 = x1 @ W1 ---
            for k in range(KO):
                nc.tensor.matmul(
                    p1_ps[:, 0:DO],
                    xk(s1_sb, k),
                    wk(s1_sb, k),
                    start=(k == 0),
                    stop=(k == KO - 1),
                )
            nc.vector.tensor_copy(out=p1bf[:], in_=p1_ps[:, 0:DO])

            # --- flatten p1 for the K=1 broadcast matmuls (Pool queue,
            # so Act.SEQ stays free for the p2 copy) ---
            for b in range(4):
                nc.gpsimd.dma_start(
                    out=p1f[32 * b : 32 * b + 1],
                    in_=p1bf[32 * b : 32 * b + 32, :],
                )

            def bcast_mm(ps, m, i):
                """psum[:, 512m:512m+DO] = broadcast of p1 row i."""
                b = i // 32
                if i < SELR:
                    nc.tensor.matmul(
                        ps[:, 512 * m : 512 * m + DO],
                        sel_sb[:, i, :],
                        p1bf[0:32, :],
                        start=True,
                        stop=True,
                        tile_position=(0, 0),
                    )
                else:
                    nc.tensor.matmul(
                        ps[:, 512 * m : 512 * m + DO],
                        ones_bf[32 * b : 32 * b + 1, 0:P],
                        p1f[32 * b : 32 * b + 1, i % 32, :],
                        start=True,
                        stop=True,
                        tile_position=(32 * b, 0),
                    )

            # g0/g1 broadcast matmuls go before the p2 matmuls on PE so
            # they run in the PE idle window while s2 streams in.
            groups = _group_rows()
            g_ps = []
            for gi in range(2):
                ps = pspool.tile([P, 512 * FUSE], F32, tag="ps", name=f"ps{gi}")
                g_ps.append(ps)
                bcast_mm(ps, 0, gi)  # groups 0/1 are single-row

            # --- p2 = x2 @ W2 + bias (bias rides a host-packed K=32
            # augmented chunk: aug[0,:P]=1, aug[0,P:]=bias, rows 1-31 zero;
            # the whole accumulation group is emitted contiguously) ---
            nc.tensor.matmul(
                p2_ps[:, 0:DO],
                aug_sb[:, 0:P],
                aug_sb[:, P : P + DO],
                start=True,
                stop=False,
                tile_position=(0, 0),
            )
            for k in range(KO):
                nc.tensor.matmul(
                    p2_ps[:, 0:DO],
                    xk(s2_sb, k),
                    wk(s2_sb, k),
                    start=False,
                    stop=(k == KO - 1),
                )
            p2_sb = cpool.tile([P, DO], F32, tag="p2")
            nc.scalar.mul(p2_sb[:], p2_ps[:, 0:DO], 1.0)
            p2v = p2_sb[:]

            # --- main loop over row groups ---
            i0 = 0
            for gi, rows in enumerate(groups):
                ob = opool.tile([P, rows, DO], F32, tag="ob", name=f"ob{gi}")
                if gi < 2:
                    ps = g_ps[gi]
                else:
                    ps = pspool.tile(
                        [P, 512 * FUSE], F32, tag="ps", name=f"ps{gi}"
                    )
                    for m in range(rows):
                        bcast_mm(ps, m, i0 + m)
                ps_v = ps.rearrange("p (i x) -> p i x", i=FUSE)[:, 0:rows, 0:DO]
                if rows == 1:
                    nc.vector.tensor_add(
                        out=ob[:, 0, :], in0=ps_v[:, 0, :], in1=p2v
                    )
                else:
                    p2_b = p2v[:, None, :].to_broadcast((P, rows, DO))
                    nc.vector.tensor_add(out=ob[:], in0=ps_v, in1=p2_b)
                dst = out_ap[i0 : i0 + rows]  # [rows, NJ, DO]
                nc.sync.dma_start(out=dst.rearrange("i j d -> j i d"), in_=ob[:])
                i0 += rows

    nc.compile()
    return nc


def _get_module():
    key = (OUT_BUFS, PSUM_BUFS, WARM_MMS)
    if key not in _cache:
        _cache[key] = _build_module()
    return _cache[key]


def _to_bf16(x):
    import ml_dtypes

    return x.astype(ml_dtypes.bfloat16)


def _prep_stream(x, Whalf):
    """x [128,768] f32, Whalf [768,384] f32 -> [128, 6*128+6*384] bf16."""
    xT = _to_bf16(x).T.reshape(KO, P, P).transpose(1, 0, 2).reshape(P, KO * P)
    Wr = (
        _to_bf16(Whalf)
        .reshape(KO, P, DO)
        .transpose(1, 0, 2)
        .reshape(P, KO * DO)
    )
    return np.ascontiguousarray(np.concatenate([xT, Wr], axis=1))


def _make_in_maps(input1, input2, W, b):
    import ml_dtypes

    input1 = np.asarray(input1, dtype=np.float32)
    input2 = np.asarray(input2, dtype=np.float32)
    W = np.asarray(W, dtype=np.float32)
    b = np.asarray(b, dtype=np.float32)

    sel0 = np.ascontiguousarray(
        np.broadcast_to(
            np.eye(32, SELR, dtype=np.float32)[:, :, None], (32, SELR, P)
        )
    ).astype(ml_dtypes.bfloat16)

    in_maps = []
    for c in range(NCORES):
        bb, h = divmod(c, 2)
        W1 = W[:KO * P, h * DO : (h + 1) * DO]
        W2 = W[KO * P :, h * DO : (h + 1) * DO]
        aug = np.zeros((32, P + DO), dtype=np.float32)
        aug[0, :P] = 1.0
        aug[0, P:] = b[h * DO : (h + 1) * DO]
        in_maps.append(
            {
                "s1": _prep_stream(input1[bb], W1),
                "s2": _prep_stream(input2[bb], W2),
                "aug": np.ascontiguousarray(_to_bf16(aug)),
                "sel0": sel0,
            }
        )
    return in_maps


def kernel(input1, input2, W, b):
    from concourse import bass_utils

    suppress_trace = False
    if os.environ.get("BASS_TRACE"):
        try:
            from antenv.axon_hooks import get_axon_ntff_profile_hook  # noqa: F401
        except Exception:
            suppress_trace = True
    prev = os.environ.get("BASS_NEVER_TRACE")
    if suppress_trace:
        os.environ["BASS_NEVER_TRACE"] = "1"
    try:
        nc = _get_module()
        in_maps = _make_in_maps(input1, input2, W, b)
        res = bass_utils.run_bass_kernel_spmd(
            nc, in_maps, core_ids=list(range(NCORES))
        )
    finally:
        if suppress_trace:
            if prev is None:
                os.environ.pop("BASS_NEVER_TRACE", None)
            else:
                os.environ["BASS_NEVER_TRACE"] = prev
    out = np.empty((4, NJ, NJ, 2 * DO), dtype=np.float32)
    for c in range(NCORES):
        bb, h = divmod(c, 2)
        out[bb, :, :, h * DO : (h + 1) * DO] = res.results[c]["out"]
    return out
